# revision 61
# baseline (speedup 1.0000x reference)
"""MiMoV2 attention (GQA + partial RoPE + attention sinks + causal) on 8 TRN2
NeuronCores.

Sharding: tensor-parallel over heads. Core c owns KV head c and query heads
[4c, 4c+4). Wq/Wk/Wv split along output dim, Wo along input dim; each core
computes a partial output [S, H] which the host sums (the Wo contraction over
heads distributes over cores).

v2: the four projection GEMMs (Q/K/V/O) run in fp8 e4m3 with DoubleRow perf
mode (2 k-subtiles contracted per instruction at 0.5 cycles/row). Precision is
recovered with a hi/lo split of BOTH operands and three product series
(hi*hi, lo*hi, hi*lo — the lo*lo term is dropped):
  x*S = x_hi + x_lo (+ ~2^-9 abs)   w*S' = w_hi + w_lo (+ ~2^-9 abs)
All three series share one PSUM accumulation group because the residuals are
stored unscaled (subnormal fp8 absolute error ~2^-10 is bf16-grade for O(1)
values). Pairing each series over adjacent k-subtiles means no operand is
duplicated: hi/lo live side by side in the layout [128, kt, 2, free] and the
DoubleRow pair dim strides over kt or over the hi/lo slot as needed.
Net PE cost: 3 x 0.5N per 2 subtiles = 0.75x bf16, with bf16-grade accuracy.

Attention (scores / exp / probs@V / ones-denominator) stays bf16: scores
contract over only d=128 (no pair dim), and probs cannot live in fp8 (exp
range without per-row max subtraction overflows e4m3).

Per-core dataflow (everything head-transposed so no on-chip transposes):
  hs8 [H, 2, S] fp8 hi/lo streamed by 128-row h-tiles; per s-chunk of 512:
    QT[d, s] (4 heads), KT[d, s] accumulate in PSUM over 16 DoubleRow pairs
    x 3 series; V[s, d] natural layout via transpose DMA of VT.
  partial RoPE applied in [d, s] layout; scale 2^-14 (hi/lo scale product)
  folded into the host-side cos/sin tensors and the pass-through copy.
  scoresT[s_k, s_q] = KT_tile^T @ QT-chunk (bf16); exp on ACT; causal via
  binary mask multiply on diagonal tiles only.
  attn_outT[d, s_q] accumulates V_tile^T @ probsT; denominator via all-ones
  stationary matmul (+ exp(sink) per-partition); at = po/den written as
  fp8 hi/lo pair (scaled x32).
  out_partial[s, o] = at-as-stationary @ Wo-chunk in fp8 DoubleRow 3-series;
  written out as bf16 scaled x2048; host sums the 8 partials in fp32 and
  divides by 2048.
"""

import numpy as np
import ml_dtypes
from contextlib import ExitStack

import concourse.bass as bass
import concourse.mybir as mybir
import concourse.tile as tile
from concourse.bass_utils import run_bass_kernel_spmd

bf16 = ml_dtypes.bfloat16
f8 = ml_dtypes.float8_e4m3
BF = mybir.dt.bfloat16
F8 = mybir.dt.float8e4
F32 = mybir.dt.float32
DR = mybir.MatmulPerfMode.DoubleRow

N_CORES = 8
S = 2048
H = 4096
HD = 128
ROPE = 64
NHL = 4                    # local query heads per core
CH = 512                   # s-chunk width
NCHUNK = S // CH           # 4
HT = H // 128              # 32 h-tiles
NPAIR = HT // 2            # 16 DoubleRow pairs over the h dimension
NKT = S // 128             # 16 k-tiles

# fp8 scale factors (powers of two; hi = fp8(S*x), lo = fp8(S*x - hi))
S_HS = 16.0                # hidden states
S_WQK = 1024.0             # Wq (with HD^-0.5 folded) and Wk
S_WV = 64.0                # Wv
S_AT = 32.0                # attention outputs
S_WO = 64.0                # Wo
QK_INV = 1.0 / (S_HS * S_WQK)   # 2^-14, folded into cos/sin + passthrough
V_INV = 1.0 / (S_HS * S_WV)     # 2^-10, applied at V copy-out
OUT_SCALE = S_AT * S_WO         # 2048, divided out on the host

# this walrus build allows at most one sync wait per instruction
_MAX_WAITS = 1


def _split_excess_waits(nc):
    cnt = 0
    for f in nc.m.functions:
        for bb in f.blocks:
            out, changed = [], False
            for inst in bb.instructions:
                si = inst.sync_info
                if si is not None and len(si.on_wait) > _MAX_WAITS:
                    waits = list(si.on_wait)
                    excess, keep = waits[:-_MAX_WAITS], waits[-_MAX_WAITS:]
                    for i in range(0, len(excess), _MAX_WAITS):
                        cnt += 1
                        out.append(mybir.InstNoOp(
                            name=f"waitnop-{cnt}", engine=inst.engine,
                            sync_info=mybir.SyncInfo(
                                on_wait=excess[i:i + _MAX_WAITS], on_update=[])))
                    si.on_wait = keep
                    changed = True
                out.append(inst)
            if changed:
                bb.instructions = out
    return cnt


def _rope_copy(nc, pool, psum_t, dest, cos_sb, sin_sb, sl):
    """psum_t [128,512] fp32 (scaled by S_HS*S_WQK) -> dest [128,512] bf16
    slice, applying partial RoPE to rows 0:64 (rotate_half = +-32-partition
    swap, sign pre-folded into sin_sb). A single ACT copy (with the 2^-14
    descale) releases the PSUM slot immediately; the rope math then runs
    in-place from SBUF (all-bf16 DVE ops, off the PSUM-release critical
    path, so PE never waits on a congested DVE queue for the next
    projection group's PSUM slot)."""
    nc.scalar.mul(dest[:, :], psum_t[:, :], QK_INV)
    # swapped copy of the rotary rows
    sw = pool.tile([64, CH], BF, tag="rope_sw")
    nc.vector.tensor_copy(sw[0:32, :], dest[32:64, :])
    nc.vector.tensor_copy(sw[32:64, :], dest[0:32, :])
    # t1 = q_r * cos   (one fused op: (dest mult 1.0) mult cos)
    t1 = pool.tile([64, CH], BF, tag="rope_t1")
    nc.vector.scalar_tensor_tensor(
        t1[:, :], dest[0:64, :], 1.0, cos_sb[:, sl],
        op0=mybir.AluOpType.mult, op1=mybir.AluOpType.mult)
    t2 = pool.tile([64, CH], BF, tag="rope_t2")
    nc.vector.tensor_mul(t2[:, :], sw[:, :], sin_sb[:, sl])
    nc.vector.tensor_add(dest[0:64, :], t1[:, :], t2[:, :])


def build_bass(repeat=1, schedule="seq"):
    """repeat>1 duplicates the whole compute body (for timing)."""
    nc = bass.Bass("TRN2", target_bir_lowering=False, debug=False)

    hs8 = nc.dram_tensor("hs8", [H, 2, S], F8, kind="ExternalInput")
    wq8 = nc.dram_tensor("wq8", [H, 2, NHL * HD], F8, kind="ExternalInput")
    wkv8 = nc.dram_tensor("wkv8", [H, 2, 2, HD], F8, kind="ExternalInput")
    wo8 = nc.dram_tensor("wo8", [NHL * HD, 2, H], F8, kind="ExternalInput")
    cosT = nc.dram_tensor("cosT", [ROPE, S], BF, kind="ExternalInput")
    sinTs = nc.dram_tensor("sinTs", [ROPE, S], BF, kind="ExternalInput")
    esink = nc.dram_tensor("esink", [NHL, 128], F32, kind="ExternalInput")
    maskb = nc.dram_tensor("maskb", [128, 1024], BF, kind="ExternalInput")
    outp = nc.dram_tensor("outp", [S, H], BF, kind="ExternalOutput")

    with tile.TileContext(nc) as tc, ExitStack() as ctx:
        const = ctx.enter_context(tc.tile_pool(name="const", bufs=1))
        hs_pool = ctx.enter_context(tc.tile_pool(name="hs", bufs=4))
        rope_pool = ctx.enter_context(tc.tile_pool(name="rope", bufs=2))
        probs_pool = ctx.enter_context(tc.tile_pool(name="probs", bufs=6))
        den_pool = ctx.enter_context(tc.tile_pool(name="den", bufs=2))
        out_pool = ctx.enter_context(tc.tile_pool(name="out", bufs=2))

        # ---- constants / weights resident in SBUF ----
        # layout [128, kt, 2(hi/lo), free]; loaded in 8-ktile slices (with a
        # small 2-ktile head slice) so the first projection matmuls only
        # wait on the first slice
        wq_sb = const.tile([128, HT, 2, NHL * HD], F8)
        # Wk/Wv interleaved [128, t, (hi/lo x k/v), HD]: the 512B-contiguous
        # per-(p,t) block keeps the weight DMAs at full rate (128B runs of a
        # separate Wk/Wv layout pay the <512B half-rate penalty right on the
        # DMA-bound chunk-0 critical path)
        wkv_sb = const.tile([128, HT, 4, HD], F8)
        wq_r = wq8.rearrange("(t p) two c -> p t two c", p=128)
        wkv_r = wkv8.rearrange("(t p) two kv c -> p t (two kv) c", p=128)
        hs_r = hs8.rearrange("(t p) two s -> p t two s", p=128)
        wo_sb = const.tile([128, NHL, 2, H], F8)
        cos_sb = const.tile([ROPE, S], BF)
        sin_sb = const.tile([ROPE, S], BF)
        mask_sb = const.tile([128, 1024], BF)
        esink_sb = const.tile([128, NHL], F32)

        def load_consts():
            # emitted after the first chunk-0 weight slices: these loads are
            # not needed until the first rope / first diagonal mask, so keep
            # them out of the latency-critical startup DMA window
            nc.gpsimd.dma_start(out=cos_sb, in_=cosT[:, :])
            nc.gpsimd.dma_start(out=sin_sb, in_=sinTs[:, :])
            nc.gpsimd.dma_start(out=mask_sb, in_=maskb[:, :])
            for h in range(NHL):
                nc.gpsimd.dma_start(out=esink_sb[:, h:h + 1],
                                  in_=esink[h].rearrange("(p c) -> p c", c=1))

        ones_sb = const.tile([128, 128], BF)
        nc.vector.memset(ones_sb[:, :], 1.0)

        # persistent activations
        qt_sb = const.tile([128, NHL, S], BF)     # QT per head [d, s]
        kt_sb = const.tile([128, S], BF)          # KT [d, s]
        vt_sb = const.tile([128, S], BF)          # VT [d, s] (pre-transpose)
        v_sb = const.tile([128, NKT, HD], BF)     # V [s(128), kt, d]
        at8_sb = const.tile([128, NHL, 2, S], F8)  # attnT hi/lo [d, s], x32

        for _rep in range(repeat):
            # phases 1+2 share one PSUM scope (8 banks: proj 2 + ps 2 + po 2
            # + pd 2) so projection chunks and attention chunks interleave on
            # PE with no pool-boundary serialization.
            with ExitStack() as p12:
                proj_pool = p12.enter_context(
                    tc.tile_pool(name="proj", bufs=3, space="PSUM"))
                ps_pool = p12.enter_context(
                    tc.tile_pool(name="ps", bufs=2, space="PSUM"))
                po_pool = p12.enter_context(
                    tc.tile_pool(name="po", bufs=2, space="PSUM"))
                pd_pool = p12.enter_context(
                    tc.tile_pool(name="pd", bufs=1, space="PSUM"))

                def emit_p1(ci, load_weights=False):
                    """QKV projections + RoPE for s-chunk ci. fp8 DoubleRow:
                    per pair pt of h-subtiles (2pt, 2pt+1), three series
                    (w_hi x hs_hi, w_hi x hs_lo, w_lo x hs_hi), all in one
                    48-matmul PSUM accumulation group per output."""
                    sl = bass.ds(ci * CH, CH)
                    hs4 = []
                    for g8 in range(HT // 8):
                        h8 = hs_pool.tile([128, 8, 2, CH], F8, tag="hst",
                                          name=f"hst_{_rep}_{ci}_{g8}")
                        # DMA APs support at most 3 dims: load hi and lo
                        # slots separately.
                        g = g8 * 8
                        if load_weights and g8 == 0:
                            # fine-grained first slices so pair 0's matmuls
                            # (hi*hi series first) start ~3us earlier
                            for w in range(2):
                                nc.sync.dma_start(out=h8[:, 0:2, w, :],
                                                  in_=hs_r[:, 0:2, w, sl])
                                nc.sync.dma_start(out=wq_sb[:, 0:2, w, :],
                                                  in_=wq_r[:, 0:2, w, :])
                                nc.sync.dma_start(
                                    out=wkv_sb[:, 0:2, 2 * w:2 * w + 2, :],
                                    in_=wkv_r[:, 0:2, 2 * w:2 * w + 2, :])
                            load_consts()
                            for w in range(2):
                                nc.sync.dma_start(out=h8[:, 2:8, w, :],
                                                  in_=hs_r[:, 2:8, w, sl])
                                nc.sync.dma_start(out=wq_sb[:, 2:8, w, :],
                                                  in_=wq_r[:, 2:8, w, :])
                                nc.sync.dma_start(
                                    out=wkv_sb[:, 2:8, 2 * w:2 * w + 2, :],
                                    in_=wkv_r[:, 2:8, 2 * w:2 * w + 2, :])
                        else:
                            for w in range(2):
                                nc.sync.dma_start(
                                    out=h8[:, :, w, :],
                                    in_=hs_r[:, g:g + 8, w, sl])
                                if load_weights:
                                    # interleave weight-slice loads with the
                                    # hst stream
                                    nc.sync.dma_start(
                                        out=wq_sb[:, g:g + 8, w, :],
                                        in_=wq_r[:, g:g + 8, w, :])
                                    nc.sync.dma_start(
                                        out=wkv_sb[:, g:g + 8,
                                                   2 * w:2 * w + 2, :],
                                        in_=wkv_r[:, g:g + 8,
                                                  2 * w:2 * w + 2, :])
                        hs4.append(h8)

                    def hsp(pt, which):
                        g8, j = pt // 4, (pt % 4) * 2
                        return hs4[g8][:, j:j + 2, which, :]

                    def copy_out(pp, rope):
                        if rope is not None:
                            _rope_copy(nc, rope_pool, pp, rope, cos_sb, sin_sb, sl)
                        else:
                            # v transposes are emitted at gen_p2(ci) start,
                            # after the next chunk's hs DMAs in the SP queue
                            nc.vector.tensor_scalar_mul(vt_sb[:, sl], pp[:, :],
                                                        V_INV)

                    groups = [
                        (lambda pt, w, h=h: wq_sb[:, 2 * pt:2 * pt + 2, w,
                                                  h * HD:(h + 1) * HD],
                         qt_sb[:, h, sl], f"q{h}") for h in range(NHL)
                    ] + [
                        (lambda pt, w: wkv_sb[:, 2 * pt:2 * pt + 2,
                                               2 * w, :],
                         kt_sb[:, sl], "k"),
                        (lambda pt, w: wkv_sb[:, 2 * pt:2 * pt + 2,
                                              2 * w + 1, :],
                         None, "v"),
                    ]

                    SERIES = [(0, 0), (0, 1), (1, 0)]  # (w slot, hs slot)

                    def mm1(pp, wf, pt, s, start, stop):
                        w_w, h_w = SERIES[s]
                        nc.tensor.matmul(pp[:, :], wf(pt, w_w), hsp(pt, h_w),
                                         start=start, stop=stop, perf_mode=DR)

                    def mm3(pp, wf, pt, start, stop):
                        mm1(pp, wf, pt, 0, start, False)
                        mm1(pp, wf, pt, 1, False, False)
                        mm1(pp, wf, pt, 2, False, stop)

                    if load_weights:
                        # chunk 0 is paced by the input DMA stream: interleave
                        # ALL 6 groups across arriving hst pairs so PE keeps
                        # up with the DMA rate; borrow the idle attention
                        # PSUM banks so all 6 run concurrently.
                        lenders = [(proj_pool, "pp"), (proj_pool, "pp"),
                                   (proj_pool, "pp"), (ps_pool, "ps"),
                                   (po_pool, "po"), (pd_pool, "pd")]
                        pps = [pool.tile([128, CH], F32, tag=tg,
                                         name=f"pp_{_rep}_{ci}_{g[2]}")
                               for (pool, tg), g in zip(lenders, groups)]
                        T0P = 12
                        # series-major within each 8-ktile group: the hi*hi
                        # matmuls need only that group's hi DMAs, hi*lo and
                        # lo*hi consume the lo DMAs as they arrive, so PE
                        # consumption order matches DMA arrival order
                        for g8 in range(T0P // 4):
                            prs = range(g8 * 4, g8 * 4 + 4)
                            for s in range(3):
                                for pt in prs:
                                    for gi in range(6):
                                        mm1(pps[gi], groups[gi][0], pt, s,
                                            start=(pt == 0 and s == 0),
                                            stop=False)
                        # ...then staggered tails so each group's copy-out
                        # chain overlaps the next group's matmuls
                        for gi in range(6):
                            for pt in range(T0P, NPAIR):
                                mm3(pps[gi], groups[gi][0], pt,
                                    start=False, stop=(pt == NPAIR - 1))
                            copy_out(pps[gi], groups[gi][1])
                    else:
                        for wf, rope, dest in groups:
                            pp = proj_pool.tile([128, CH], F32,
                                                name=f"pp_{_rep}_{ci}_{dest}",
                                                tag="pp")
                            for pt in range(NPAIR):
                                mm3(pp, wf, pt, start=(pt == 0),
                                    stop=(pt == NPAIR - 1))
                            copy_out(pp, rope)

                def gen_p1(ci):
                    """Generator form of emit_p1 (no weight loads): yields
                    after each of the 6 projection groups so attention heads
                    can be woven between them."""
                    sl = bass.ds(ci * CH, CH)
                    hs4 = []
                    for g8 in range(HT // 8):
                        h8 = hs_pool.tile([128, 8, 2, CH], F8, tag="hst",
                                          name=f"hst_{_rep}_{ci}_{g8}")
                        for w in range(2):
                            nc.sync.dma_start(
                                out=h8[:, :, w, :],
                                in_=hs_r[:, g8 * 8:(g8 + 1) * 8, w, sl])
                        hs4.append(h8)

                    def hsp(pt, which):
                        g8, j = pt // 4, (pt % 4) * 2
                        return hs4[g8][:, j:j + 2, which, :]

                    def copy_out(pp, rope):
                        if rope is not None:
                            _rope_copy(nc, rope_pool, pp, rope, cos_sb, sin_sb, sl)
                        else:
                            # v transposes are emitted at gen_p2(ci) start,
                            # after the next chunk's hs DMAs in the SP queue
                            nc.vector.tensor_scalar_mul(vt_sb[:, sl], pp[:, :],
                                                        V_INV)

                    groups = [
                        (lambda pt, w, h=h: wq_sb[:, 2 * pt:2 * pt + 2, w,
                                                  h * HD:(h + 1) * HD],
                         qt_sb[:, h, sl], f"q{h}") for h in range(NHL)
                    ] + [
                        (lambda pt, w: wkv_sb[:, 2 * pt:2 * pt + 2,
                                               2 * w, :],
                         kt_sb[:, sl], "k"),
                        (lambda pt, w: wkv_sb[:, 2 * pt:2 * pt + 2,
                                              2 * w + 1, :],
                         None, "v"),
                    ]
                    for wf, rope, dest in groups:
                        pp = proj_pool.tile([128, CH], F32,
                                            name=f"pp_{_rep}_{ci}_{dest}",
                                            tag="pp")
                        for pt in range(NPAIR):
                            nc.tensor.matmul(pp[:, :], wf(pt, 0), hsp(pt, 0),
                                             start=(pt == 0), stop=False,
                                             perf_mode=DR)
                            nc.tensor.matmul(pp[:, :], wf(pt, 0), hsp(pt, 1),
                                             start=False, stop=False,
                                             perf_mode=DR)
                            nc.tensor.matmul(pp[:, :], wf(pt, 1), hsp(pt, 0),
                                             start=False,
                                             stop=(pt == NPAIR - 1),
                                             perf_mode=DR)
                        copy_out(pp, rope)
                        yield

                def gen_p2(ci):
                    """Attention for query chunk ci (bf16); yields after each
                    of the 4 local heads. Emission is software-pipelined:
                    scores(kj+1) is emitted before attnV(kj) so PE computes
                    the next score tile while ACT does exp of the previous
                    one."""
                    q0 = ci * CH
                    n_kt = 4 * (ci + 1)
                    for st in range(4):
                        kj = ci * 4 + st
                        nc.sync.dma_start_transpose(
                            out=v_sb[:, kj, :],
                            in_=vt_sb[:, kj * 128:(kj + 1) * 128])
                    for h in range(NHL):
                        po = po_pool.tile([128, CH], F32,
                                          name=f"po_{_rep}_{ci}_{h}", tag="po")
                        pd = pd_pool.tile([128, CH], F32,
                                          name=f"pd_{_rep}_{ci}_{h}", tag="pd")
                        stage = []  # (kj, ps, pr, off)

                        def emit_scores(kj):
                            off = kj * 128 - q0
                            ps = ps_pool.tile([128, CH], F32,
                                              name=f"ps_{_rep}_{ci}_{h}_{kj}",
                                              tag="ps")
                            kt_t = kt_sb[:, kj * 128:(kj + 1) * 128]
                            if off > 0:
                                # columns < off are fully masked: skip them
                                nc.tensor.matmul(ps[:, off:],
                                                 kt_t, qt_sb[:, h, q0 + off:q0 + CH],
                                                 start=True, stop=True)
                            else:
                                nc.tensor.matmul(ps[:, :], kt_t,
                                                 qt_sb[:, h, q0:q0 + CH],
                                                 start=True, stop=True)
                            pr = probs_pool.tile([128, CH], BF,
                                                 name=f"pr_{_rep}_{ci}_{h}_{kj}",
                                                 tag="pr")
                            if off > 0:
                                nc.gpsimd.memset(pr[:, 0:off], 0.0)
                                nc.scalar.activation(
                                    pr[:, off:], ps[:, off:],
                                    mybir.ActivationFunctionType.Exp)
                            else:
                                nc.scalar.activation(
                                    pr[:, :], ps[:, :],
                                    mybir.ActivationFunctionType.Exp)
                            if off >= 0:
                                # triangular 128-col band at q_local in
                                # [off, off+128): maskb[:, 512:640] is the
                                # aligned triangle for every diagonal tile.
                                # On gpsimd: keeps the DVE queue free of
                                # exp-dependent ops (head-of-line blocking).
                                nc.gpsimd.tensor_mul(
                                    pr[:, off:off + 128], pr[:, off:off + 128],
                                    mask_sb[:, 512:640])
                            stage.append((kj, ps, pr, off))

                        def emit_av():
                            kj, ps, pr, off = stage.pop(0)
                            fl = dict(start=(kj == 0), stop=(kj == n_kt - 1))
                            if off > 0:
                                nc.tensor.matmul(po[:, off:], v_sb[:, kj, :],
                                                 pr[:, off:], **fl)
                                nc.tensor.matmul(pd[:, off:], ones_sb[:, :],
                                                 pr[:, off:], **fl)
                            else:
                                nc.tensor.matmul(po[:, :], v_sb[:, kj, :],
                                                 pr[:, :], **fl)
                                nc.tensor.matmul(pd[:, :], ones_sb[:, :],
                                                 pr[:, :], **fl)

                        emit_scores(0)
                        for kj in range(1, n_kt):
                            emit_scores(kj)
                            emit_av()
                        emit_av()

                        den = den_pool.tile([128, CH], F32, tag="den",
                                            name=f"den_{_rep}_{ci}_{h}")
                        nc.vector.tensor_scalar_add(den[:, :], pd[:, :],
                                                    esink_sb[:, h:h + 1])
                        rec = den_pool.tile([128, CH], F32, tag="rec",
                                            name=f"rec_{_rep}_{ci}_{h}")
                        nc.vector.reciprocal(rec[:, :], den[:, :])
                        # at_hi = fp8(32*po/den); at_lo = fp8(32*po/den - hi)
                        tmp = den_pool.tile([128, CH], BF, tag="attmp",
                                            name=f"attmp_{_rep}_{ci}_{h}")
                        nc.vector.scalar_tensor_tensor(
                            tmp[:, :], po[:, :], S_AT, rec[:, :],
                            op0=mybir.AluOpType.mult, op1=mybir.AluOpType.mult)
                        # hi on DVE too: keeps the whole den/rec/hi/lo chain
                        # on one engine (no cross-engine ping-pong with exp)
                        nc.vector.tensor_copy(at8_sb[:, h, 0, q0:q0 + CH],
                                              tmp[:, :])
                        nc.vector.scalar_tensor_tensor(
                            at8_sb[:, h, 1, q0:q0 + CH],
                            at8_sb[:, h, 0, q0:q0 + CH], -1.0, tmp[:, :],
                            op0=mybir.AluOpType.mult, op1=mybir.AluOpType.add)
                        yield

                def gen_p3(ci, pool, bufs_tag):
                    """Output projection for the 4 s-tiles of chunk ci, fp8
                    DoubleRow over head-subtile pairs (0,1) and (2,3).
                    Yields after each s-tile."""
                    for st in range(ci * 4, (ci + 1) * 4):
                        ob = out_pool.tile([128, H], BF, tag="ob",
                                           name=f"ob_{_rep}_{st}")
                        sts = bass.ds(st * 128, 128)
                        for oc in range(H // CH):
                            pw = pool.tile([128, CH], F32, tag=bufs_tag,
                                           name=f"pw_{_rep}_{st}_{oc}")
                            ocs = bass.ds(oc * CH, CH)
                            for g in (0, 2):
                                a_hi = at8_sb[:, g:g + 2, 0, sts]
                                a_lo = at8_sb[:, g:g + 2, 1, sts]
                                w_hi = wo_sb[:, g:g + 2, 0, ocs]
                                w_lo = wo_sb[:, g:g + 2, 1, ocs]
                                nc.tensor.matmul(pw[:, :], a_hi, w_hi,
                                                 start=(g == 0), stop=False,
                                                 perf_mode=DR)
                                nc.tensor.matmul(pw[:, :], a_lo, w_hi,
                                                 start=False, stop=False,
                                                 perf_mode=DR)
                                nc.tensor.matmul(pw[:, :], a_hi, w_lo,
                                                 start=False, stop=(g == 2),
                                                 perf_mode=DR)
                            # alternate copy engine to split PSUM->SBUF load
                            if (st * (H // CH) + oc) % 2 == 0:
                                nc.vector.tensor_copy(
                                    ob[:, oc * CH:(oc + 1) * CH], pw[:, :])
                            else:
                                nc.scalar.copy(
                                    ob[:, oc * CH:(oc + 1) * CH], pw[:, :])
                            if ci == NCHUNK - 1 and oc % 2 == 1:
                                # last chunk: store in quarters as soon as
                                # each pair of copies lands, shortening the
                                # end-of-kernel DMA tail
                                c0 = (oc - 1) * CH
                                nc.sync.dma_start(
                                    out=outp[st * 128:(st + 1) * 128,
                                             c0:c0 + 2 * CH],
                                    in_=ob[:, c0:c0 + 2 * CH])
                        if ci != NCHUNK - 1:
                            nc.sync.dma_start(
                                out=outp[st * 128:(st + 1) * 128, :],
                                in_=ob[:, :])
                        yield

                def weave(*gens):
                    """Round-robin the generators (one unit each per turn)
                    until all are exhausted."""
                    live = list(gens)
                    while live:
                        for g in list(live):
                            try:
                                next(g)
                            except StopIteration:
                                live.remove(g)

                # Schedule: during attention phases the ACT engine (exp) is
                # rate-matched with PE, so PE stalls whenever ACT hiccups.
                # Weaving projection groups / output-projection s-tiles
                # between attention heads gives PE exp-independent filler
                # while ACT catches up.
                def run(g):
                    for _ in g:
                        pass

                wo_r = wo8.rearrange("(t p) two c -> p t two c", p=128)

                def load_wo():
                    for w in range(2):
                        nc.sync.dma_start(out=wo_sb[:, 0:2, w, :],
                                          in_=wo_r[:, 0:2, w, :])
                        nc.sync.dma_start(out=wo_sb[:, 2:4, w, :],
                                          in_=wo_r[:, 2:4, w, :])

                if _rep == 0:
                    # warm-up: keep PE continuously busy through the initial
                    # weight-DMA wait so the HAM/pstate ramp completes before
                    # the first real matmul
                    wrm = proj_pool.tile([128, 64], F32, tag="pp",
                                         name="warmup")
                    for i in range(24):
                        nc.tensor.matmul(wrm[:, :], ones_sb[:, 0:128],
                                         ones_sb[:, 0:64],
                                         start=(i == 0), stop=(i == 23))
                emit_p1(0, load_weights=(_rep == 0))
                if schedule == "weave":
                    run(gen_p1(1))
                    weave(gen_p2(0), gen_p1(2))
                    if _rep == 0:
                        load_wo()
                    weave(gen_p2(1), gen_p1(3))
                    weave(gen_p2(2), gen_p3(0, proj_pool, "pp"))
                    weave(gen_p2(3), gen_p3(1, proj_pool, "pp"))
                    run(gen_p3(2, proj_pool, "pp"))
                    run(gen_p3(3, proj_pool, "pp"))
                else:
                    run(gen_p1(1))
                    run(gen_p2(0))
                    run(gen_p1(2))
                    if _rep == 0:
                        load_wo()
                    run(gen_p2(1))
                    run(gen_p3(0, proj_pool, "pp"))
                    run(gen_p1(3))
                    run(gen_p2(2))
                    run(gen_p3(1, proj_pool, "pp"))
                    run(gen_p2(3))
                    run(gen_p3(2, proj_pool, "pp"))
                    run(gen_p3(3, proj_pool, "pp"))

    _split_excess_waits(nc)
    return nc


_NC_CACHE = None


def _get_nc():
    global _NC_CACHE
    if _NC_CACHE is None:
        _NC_CACHE = build_bass()
    return _NC_CACHE


def _split8(x, scale):
    """x (fp32) -> (hi, lo) fp8 e4m3 arrays of scale*x."""
    xs = (x * scale).astype(np.float32)
    hi = xs.astype(f8)
    lo = (xs - hi.astype(np.float32)).astype(f8)
    return hi, lo


def _hilo(x, scale, axis):
    hi, lo = _split8(x, scale)
    return np.ascontiguousarray(np.stack([hi, lo], axis=axis))


def make_in_maps(hidden_states, cos, sin, Wq, Wk, Wv, Wo, sinks):
    scaling = HD ** -0.5
    hs = np.asarray(hidden_states, dtype=np.float32).reshape(S, H)
    hsT = np.ascontiguousarray(hs.T)
    hs8 = _hilo(hsT, S_HS, axis=1)                      # [H, 2, S]
    cosT = np.ascontiguousarray(np.asarray(cos, np.float32).reshape(S, ROPE).T)
    sinT = np.ascontiguousarray(np.asarray(sin, np.float32).reshape(S, ROPE).T)
    sinTs = sinT.copy()
    sinTs[:ROPE // 2] *= -1.0
    cosT = cosT.astype(bf16)
    sinTs = sinTs.astype(bf16)
    Wq = np.asarray(Wq, np.float32)
    Wk = np.asarray(Wk, np.float32)
    Wv = np.asarray(Wv, np.float32)
    Wo = np.asarray(Wo, np.float32)
    sinks = np.asarray(sinks, np.float32)
    maskb = ((np.arange(1024)[None, :] - 512) >= np.arange(128)[:, None])
    maskb = maskb.astype(np.float32).astype(bf16)

    in_maps = []
    for c in range(N_CORES):
        qcols = slice(NHL * HD * c, NHL * HD * (c + 1))
        esink_c = np.exp(sinks[NHL * c:NHL * (c + 1)]).astype(np.float32)
        in_maps.append({
            "hs8": hs8,
            "wq8": _hilo(Wq[:, qcols] * scaling, S_WQK, axis=1),
            "wkv8": np.ascontiguousarray(np.stack([
                np.stack(_split8(Wk[:, HD * c:HD * (c + 1)], S_WQK), axis=0),
                np.stack(_split8(Wv[:, HD * c:HD * (c + 1)], S_WV), axis=0),
            ], axis=2).transpose(1, 0, 2, 3)),
            "wo8": _hilo(Wo[qcols, :], S_WO, axis=1),
            "cosT": cosT,
            "sinTs": sinTs,
            "esink": np.repeat(esink_c[:, None], 128, axis=1).copy(),
            "maskb": maskb,
        })
    return in_maps


def kernel(hidden_states, cos, sin, attention_mask, Wq, Wk, Wv, Wo, sinks):
    # attention_mask is the standard causal mask; causality is built into the
    # kernel (binary masks on the diagonal score tiles), so it is unused.
    in_maps = make_in_maps(hidden_states, cos, sin, Wq, Wk, Wv, Wo, sinks)
    nc = _get_nc()
    res = run_bass_kernel_spmd(nc, in_maps, core_ids=list(range(N_CORES)))
    acc = np.zeros((S, H), dtype=np.float32)
    for r in res.results:
        acc += r["outp"].astype(np.float32)
    acc *= 1.0 / OUT_SCALE
    return acc.reshape(1, S, H)
